# revision 5
# baseline (speedup 1.0000x reference)
"""Multi-head causal attention (B=4, S=2048, E=1024, H=16, D=64) on 8 trn2 cores.

Sharding: core c handles batch b = c//2 and head-group g = c%2 (8 heads each).
Each core computes its partial output projection over its 512 local concat
columns; the host sums the two partials per batch and adds bp.

Layout strategy (per core):
  - x is pre-transposed on host: xT_aug [1025(+pad), S] with a ones row so the
    V bias rides along the matmul.
  - Q^T, K^T computed as [d, s] (d on partitions, 2 heads per 128-partition
    pair tile) so scores come out transposed: scoresT [t, s].
  - V kept natural [t, d] with a ones column appended per head (padded to 66
    cols for fp32r even-stride rules), so the PV matmul also produces the
    softmax denominator as row 64 of its output.
  - Softmax: exp on ACT (no max subtraction -- scores are O(1) by
    construction), causal masking via multiplicative 0/1 masks on DVE,
    denominator broadcast across partitions via a K=1 matmul, reciprocal on
    DVE, normalize into concat^T, output projection directly from concat^T.

MM_DTYPE picks the matmul input precision: float32 (exact, 4 cyc/row),
float32r (TF32, 1 cyc/row at N>=512), bfloat16 (1 cyc/row).
"""

import numpy as np

B, S, E, H, D = 4, 2048, 1024, 16, 64
NCORES = 8
PAIRS = 4  # head pairs per core (8 heads)
ET = 8  # e-tiles of 128 for the contraction over E
SCH = 4  # s-chunks of 512
VW = 66  # V columns per head: 64 d + 1 ones + 1 pad
SCALE = float(D) ** -0.5

MM_DTYPE = "float32r"

_CACHE = {}


def _round_tf32(a):
    b = np.ascontiguousarray(a, np.float32).view(np.uint32).copy()
    b += 0x0FFF + ((b >> 13) & 1)  # round-to-nearest-even into 19 bits
    b &= np.uint32(0xFFFFE000)
    return b.view(np.float32)


def host_round(a):
    a = np.ascontiguousarray(a, np.float32)
    if MM_DTYPE == "float32r":
        return _round_tf32(a)
    if MM_DTYPE == "bfloat16":
        import ml_dtypes

        return a.astype(ml_dtypes.bfloat16)
    return a


def _build():
    import concourse.tile as tile
    from concourse import bacc, mybir
    from contextlib import ExitStack

    f32 = mybir.dt.float32
    mdt = getattr(mybir.dt, MM_DTYPE)
    AF = mybir.ActivationFunctionType

    nc = bacc.Bacc("TRN2", target_bir_lowering=False, debug=False, num_devices=NCORES)

    xt_d = nc.dram_tensor("xt", [9, 128, S], mdt, kind="ExternalInput").ap()
    wq_d = nc.dram_tensor("wq", [128, ET, 512], mdt, kind="ExternalInput").ap()
    wk_d = nc.dram_tensor("wk", [128, ET, 512], mdt, kind="ExternalInput").ap()
    wv_d = nc.dram_tensor("wv", [128, 9, 512], mdt, kind="ExternalInput").ap()
    wp_d = nc.dram_tensor("wp", [128, PAIRS, E], mdt, kind="ExternalInput").ap()
    bq_d = nc.dram_tensor("bq", [128, PAIRS], f32, kind="ExternalInput").ap()
    bk_d = nc.dram_tensor("bk", [128, PAIRS], f32, kind="ExternalInput").ap()
    mask_d = nc.dram_tensor("mask", [128, 4, 512], mdt, kind="ExternalInput").ap()
    y_d = nc.dram_tensor("y", [S, E], f32, kind="ExternalOutput").ap()
    cat_d = nc.dram_tensor("cat_scratch", [PAIRS, 128, S], mdt).ap()

    with tile.TileContext(nc) as tc, ExitStack() as ctx:
        pers = ctx.enter_context(tc.tile_pool(name="pers", bufs=1))
        qt = pers.tile([128, PAIRS, S], mdt)  # Q^T pair tiles
        kt = pers.tile([128, PAIRS, S], mdt)  # K^T pair tiles
        va = pers.tile([128, 16, 8 * VW], mdt)  # V (+ones col) per t-block
        bq_sb = pers.tile([128, PAIRS], f32)
        bk_sb = pers.tile([128, PAIRS], f32)
        ones65 = pers.tile([65, 64], mdt)
        nc.sync.dma_start(out=bq_sb, in_=bq_d)
        nc.sync.dma_start(out=bk_sb, in_=bk_d)
        nc.vector.memset(ones65.bitcast(mybir.dt.uint32), 0x3F800000)

        # ---------------- Phase 1: QKV projections ----------------
        with tc.tile_pool(name="ph1", bufs=1) as p1, tc.tile_pool(
            name="ps1", bufs=1, space="PSUM"
        ) as ps1:
            wq_sb = p1.tile([128, ET, 512], mdt, bufs=1)
            wk_sb = p1.tile([128, ET, 512], mdt, bufs=1)
            wv_sb = p1.tile([128, 9, 512], mdt, bufs=1)
            nc.sync.dma_start(out=wq_sb, in_=wq_d)
            nc.sync.dma_start(out=wk_sb, in_=wk_d)
            nc.sync.dma_start(out=wv_sb, in_=wv_d)
            xt_view = xt_d.rearrange("e p s -> p e s")
            for j in range(SCH):
                sj = slice(j * 512, (j + 1) * 512)
                xt_t = p1.tile([128, 9, 512], mdt, tag="xt", bufs=2)
                nc.sync.dma_start(out=xt_t, in_=xt_view[:, :, sj])
                # Q^T and K^T: [d_pair, s] per pair
                for r in range(PAIRS):
                    for w_sb, dst, b_sb in (
                        (wq_sb, qt, bq_sb),
                        (wk_sb, kt, bk_sb),
                    ):
                        ps = ps1.tile([128, 512], f32, tag="qkv", bufs=4)
                        for et in range(ET):
                            nc.tensor.matmul(
                                ps,
                                lhsT=w_sb[:, et, r * 128 : (r + 1) * 128],
                                rhs=xt_t[:, et, :],
                                start=(et == 0),
                                stop=(et == ET - 1),
                            )
                        nc.vector.tensor_scalar_add(
                            dst[:, r, sj], ps, b_sb[:, r : r + 1]
                        )
                # V natural [t, d] for all 8 heads at once (+bias via ones row)
                for ii in range(4):
                    i = 4 * j + ii
                    si = slice(ii * 128, (ii + 1) * 128)
                    ps = ps1.tile([128, 512], f32, tag="qkv", bufs=4)
                    for et in range(ET):
                        nc.tensor.matmul(
                            ps,
                            lhsT=xt_t[:, et, si],
                            rhs=wv_sb[:, et, :],
                            start=(et == 0),
                            stop=False,
                        )
                    nc.tensor.matmul(
                        ps,
                        lhsT=xt_t[0:1, 8, si],
                        rhs=wv_sb[0:1, 8, :],
                        start=False,
                        stop=True,
                    )
                    va_i = va[:, i, :].rearrange("p (h c) -> p h c", c=VW)
                    nc.vector.tensor_copy(
                        va_i[:, :, 0:64], ps.rearrange("p (h d) -> p h d", d=64)
                    )
                    nc.vector.memset(
                        va_i[:, :, 64:65].bitcast(mybir.dt.uint32), 0x3F800000
                    )
                    nc.vector.memset(
                        va_i[:, :, 65:66].bitcast(mybir.dt.uint32), 0
                    )

        # ---------------- Phase 2: causal attention ----------------
        with tc.tile_pool(name="ph2", bufs=1) as p2, tc.tile_pool(
            name="ps2", bufs=1, space="PSUM"
        ) as ps2:
            mask_sb = p2.tile([128, 4, 512], mdt, bufs=1)
            nc.sync.dma_start(out=mask_sb, in_=mask_d)
            for r in range(PAIRS):
                for j in range(SCH):
                    sj = slice(j * 512, (j + 1) * 512)
                    nt = 4 * j + 4  # causal: t-blocks 0 .. 4j+3
                    outps = [
                        ps2.tile([VW, 512], f32, tag=f"o{hh}", bufs=1,
                                 name=f"outp{hh}")
                        for hh in range(2)
                    ]
                    for ti in range(nt):
                        tis = slice(ti * 128, (ti + 1) * 128)
                        prs = []
                        for hh in range(2):
                            po = hh * 64
                            scp = ps2.tile(
                                [128, 512], f32, tag=f"sc{hh}", bufs=2
                            )
                            nc.tensor.matmul(
                                scp,
                                lhsT=kt[po : po + 64, r, tis],
                                rhs=qt[po : po + 64, r, sj],
                                start=True,
                                stop=True,
                            )
                            pr = p2.tile([128, 512], mdt, tag=f"pr{hh}", bufs=3)
                            nc.scalar.activation(pr, scp, AF.Exp, scale=SCALE)
                            if ti >= 4 * j:
                                nc.vector.tensor_mul(
                                    pr, pr, mask_sb[:, ti - 4 * j, :]
                                )
                            prs.append(pr)
                        for hh in range(2):
                            h = 2 * r + hh
                            nc.tensor.matmul(
                                outps[hh],
                                lhsT=va[:, ti, h * VW : (h + 1) * VW],
                                rhs=prs[hh],
                                start=(ti == 0),
                                stop=(ti == nt - 1),
                            )
                    # normalize and write concat^T to DRAM scratch
                    for hh in range(2):
                        po = hh * 64
                        dn = p2.tile([65, 512], mdt, tag="dn", bufs=2)
                        nc.vector.tensor_copy(dn[64:65, :], outps[hh][64:65, :])
                        bc = ps2.tile([64, 512], f32, tag="bc", bufs=2)
                        nc.tensor.matmul(
                            bc,
                            lhsT=ones65[64:65, :],
                            rhs=dn[64:65, :],
                            start=True,
                            stop=True,
                        )
                        rd = p2.tile([64, 512], f32, tag="rd", bufs=2)
                        rs = p2.tile([64, 512], f32, tag="rs", bufs=2)
                        nc.vector.reciprocal_approx_accurate(rd, bc, scratch=rs)
                        cw = p2.tile([64, 512], mdt, tag="cw", bufs=3)
                        nc.vector.tensor_mul(cw, outps[hh][0:64, :], rd)
                        nc.sync.dma_start(
                            out=cat_d[r, po : po + 64, sj], in_=cw
                        )

        # ---------------- Phase 3: output projection (partial) ----------------
        with tc.tile_pool(name="ph3", bufs=1) as p3, tc.tile_pool(
            name="ps3", bufs=1, space="PSUM"
        ) as ps3:
            wp_sb = p3.tile([128, PAIRS, E], mdt, bufs=1)
            nc.sync.dma_start(out=wp_sb, in_=wp_d)
            cat_view = cat_d.rearrange("r p s -> p r s")
            for sb in range(16):
                ss = slice(sb * 128, (sb + 1) * 128)
                ct = p3.tile([128, PAIRS, 128], mdt, tag="ct", bufs=3)
                nc.sync.dma_start(out=ct, in_=cat_view[:, :, ss])
                for f in range(2):
                    sf = slice(f * 512, (f + 1) * 512)
                    yp = ps3.tile([128, 512], f32, tag="y", bufs=4)
                    for r in range(PAIRS):
                        nc.tensor.matmul(
                            yp,
                            lhsT=ct[:, r, :],
                            rhs=wp_sb[:, r, sf],
                            start=(r == 0),
                            stop=(r == PAIRS - 1),
                        )
                    ys = p3.tile([128, 512], f32, tag="ys", bufs=3)
                    nc.vector.tensor_copy(ys, yp)
                    nc.sync.dma_start(out=y_d[ss, sf], in_=ys)

    nc.compile()
    return nc


def get_nc():
    if "nc" not in _CACHE:
        _CACHE["nc"] = _build()
    return _CACHE["nc"]


def prep_core_inputs(x, Wq, bq, Wk, bk, Wv, bv, Wp, core):
    """Pack the full-model inputs into one core's input map."""
    b, g = core // 2, core % 2
    heads = list(range(g * 8, g * 8 + 8))

    xa = np.zeros((9 * 128, S), np.float32)
    xa[:E] = x[b].T
    xa[E] = 1.0

    def pack_w(W):  # [H,E,D] -> local [E, 512] -> [128, 8, 512]
        Wl = np.concatenate([W[h] for h in heads], axis=1)
        return host_round(Wl.reshape(ET, 128, 512).transpose(1, 0, 2))

    wv_aug = np.zeros((9 * 128, 512), np.float32)
    wv_aug[:E] = np.concatenate([Wv[h] for h in heads], axis=1)
    wv_aug[E] = np.concatenate([bv[h] for h in heads])

    wp_l = host_round(
        Wp[g * 512 : (g + 1) * 512].reshape(PAIRS, 128, E).transpose(1, 0, 2)
    )

    def pack_b(bias):
        return np.stack(
            [
                np.concatenate([bias[heads[2 * r]], bias[heads[2 * r + 1]]])
                for r in range(PAIRS)
            ],
            axis=1,
        ).astype(np.float32)

    p = np.arange(128)[:, None, None]
    v = np.arange(4)[None, :, None]
    c = np.arange(512)[None, None, :]
    mask = host_round((c >= p + 128 * v).astype(np.float32))

    return {
        "xt": host_round(xa.reshape(9, 128, S)),
        "wq": pack_w(Wq),
        "wk": pack_w(Wk),
        "wv": host_round(wv_aug.reshape(9, 128, 512).transpose(1, 0, 2)),
        "wp": wp_l,
        "bq": pack_b(bq),
        "bk": pack_b(bk),
        "mask": mask,
    }


def kernel(**inputs):
    from concourse.bass_utils import run_bass_kernel_spmd

    args = {k: np.asarray(v, np.float32) for k, v in inputs.items()}
    nc = get_nc()
    in_maps = [
        prep_core_inputs(
            args["x"], args["Wq"], args["bq"], args["Wk"], args["bk"],
            args["Wv"], args["bv"], args["Wp"], c,
        )
        for c in range(NCORES)
    ]
    res = run_bass_kernel_spmd(nc, in_maps, core_ids=list(range(NCORES)))
    parts = [r["y"] for r in res.results]
    out = np.stack([parts[2 * b] + parts[2 * b + 1] for b in range(B)])
    return (out + args["bp"][None, None, :]).astype(np.float32)


# revision 7
# speedup vs baseline: 1.2484x; 1.2484x over previous
"""Multi-head causal attention (B=4, S=2048, E=1024, H=16, D=64) on 8 trn2 cores.

Sharding: core c handles batch b = c//2 and head-group g = c%2 (8 heads each).
Each core computes its partial output projection over its 512 local concat
columns; the host sums the two partials per batch and adds bp.

Layout strategy (per core):
  - x is pre-transposed on host: xT_aug [1025(+pad), S] with a ones row so the
    V bias rides along the matmul.
  - Q^T, K^T computed as [d, s] (d on partitions, 2 heads per 128-partition
    pair tile) so scores come out transposed: scoresT [t, s].
  - V kept natural [t, d] with a ones column appended per head (padded to 66
    cols for fp32r even-stride rules), so the PV matmul also produces the
    softmax denominator as row 64 of its output.
  - Softmax: exp on ACT (no max subtraction -- scores are O(1) by
    construction), causal masking via multiplicative 0/1 masks on DVE,
    denominator broadcast across partitions via a K=1 matmul, reciprocal on
    DVE, normalize into concat^T, output projection directly from concat^T.

MM_DTYPE picks the matmul input precision: float32 (exact, 4 cyc/row),
float32r (TF32, 1 cyc/row at N>=512), bfloat16 (1 cyc/row).
"""

import numpy as np

B, S, E, H, D = 4, 2048, 1024, 16, 64
NCORES = 8
PAIRS = 4  # head pairs per core (8 heads)
ET = 8  # e-tiles of 128 for the contraction over E
SCH = 4  # s-chunks of 512
VW = 66  # V columns per head: 64 d + 1 ones + 1 pad
SCALE = float(D) ** -0.5

MM_DTYPE = "bfloat16"

_CACHE = {}


def _round_tf32(a):
    b = np.ascontiguousarray(a, np.float32).view(np.uint32).copy()
    b += 0x0FFF + ((b >> 13) & 1)  # round-to-nearest-even into 19 bits
    b &= np.uint32(0xFFFFE000)
    return b.view(np.float32)


def host_round(a):
    a = np.ascontiguousarray(a, np.float32)
    if MM_DTYPE == "float32r":
        return _round_tf32(a)
    if MM_DTYPE == "bfloat16":
        import ml_dtypes

        return a.astype(ml_dtypes.bfloat16)
    return a


def _build():
    import concourse.tile as tile
    from concourse import bacc, mybir
    from contextlib import ExitStack

    f32 = mybir.dt.float32
    mdt = getattr(mybir.dt, MM_DTYPE)
    AF = mybir.ActivationFunctionType

    nc = bacc.Bacc("TRN2", target_bir_lowering=False, debug=False, num_devices=NCORES)

    xt_d = nc.dram_tensor("xt", [9, 128, S], mdt, kind="ExternalInput").ap()
    wq_d = nc.dram_tensor("wq", [128, ET, 512], mdt, kind="ExternalInput").ap()
    wk_d = nc.dram_tensor("wk", [128, ET, 512], mdt, kind="ExternalInput").ap()
    wv_d = nc.dram_tensor("wv", [128, 9, 512], mdt, kind="ExternalInput").ap()
    wp_d = nc.dram_tensor("wp", [128, PAIRS, E], mdt, kind="ExternalInput").ap()
    bq_d = nc.dram_tensor("bq", [128, PAIRS], f32, kind="ExternalInput").ap()
    bk_d = nc.dram_tensor("bk", [128, PAIRS], f32, kind="ExternalInput").ap()
    mask_d = nc.dram_tensor("mask", [128, 4, 512], mdt, kind="ExternalInput").ap()
    y_d = nc.dram_tensor("y", [S, E], f32, kind="ExternalOutput").ap()
    cat_d = nc.dram_tensor("cat_scratch", [PAIRS, 128, S], mdt).ap()

    with tile.TileContext(nc) as tc, ExitStack() as ctx:
        pers = ctx.enter_context(tc.tile_pool(name="pers", bufs=1))
        qt = pers.tile([128, PAIRS, S], mdt)  # Q^T pair tiles
        kt = pers.tile([128, PAIRS, S], mdt)  # K^T pair tiles
        va = pers.tile([128, 16, 8 * VW], mdt)  # V (+ones col) per t-block
        bq_sb = pers.tile([128, PAIRS], f32)
        bk_sb = pers.tile([128, PAIRS], f32)
        ones65 = pers.tile([65, 64], mdt)
        nc.sync.dma_start(out=bq_sb, in_=bq_d)
        nc.sync.dma_start(out=bk_sb, in_=bk_d)
        if MM_DTYPE == "float32r":
            nc.vector.memset(ones65.bitcast(mybir.dt.uint32), 0x3F800000)
        else:
            nc.vector.memset(ones65, 1.0)

        # ---------------- Phase 1: QKV projections ----------------
        with tc.tile_pool(name="ph1", bufs=1) as p1, tc.tile_pool(
            name="ps1", bufs=1, space="PSUM"
        ) as ps1:
            wq_sb = p1.tile([128, ET, 512], mdt, bufs=1)
            wk_sb = p1.tile([128, ET, 512], mdt, bufs=1)
            wv_sb = p1.tile([128, 9, 512], mdt, bufs=1)
            nc.sync.dma_start(out=wq_sb, in_=wq_d)
            nc.sync.dma_start(out=wk_sb, in_=wk_d)
            nc.sync.dma_start(out=wv_sb, in_=wv_d)
            xt_view = xt_d.rearrange("e p s -> p e s")
            for j in range(SCH):
                sj = slice(j * 512, (j + 1) * 512)
                xt_t = p1.tile([128, 9, 512], mdt, tag="xt", bufs=2)
                nc.sync.dma_start(out=xt_t, in_=xt_view[:, :, sj])
                # Q^T and K^T: [d_pair, s] per pair
                for r in range(PAIRS):
                    for w_sb, dst, b_sb in (
                        (wq_sb, qt, bq_sb),
                        (wk_sb, kt, bk_sb),
                    ):
                        ps = ps1.tile([128, 512], f32, tag="qkv", bufs=4)
                        for et in range(ET):
                            nc.tensor.matmul(
                                ps,
                                lhsT=w_sb[:, et, r * 128 : (r + 1) * 128],
                                rhs=xt_t[:, et, :],
                                start=(et == 0),
                                stop=(et == ET - 1),
                            )
                        nc.vector.tensor_scalar_add(
                            dst[:, r, sj], ps, b_sb[:, r : r + 1]
                        )
                # V natural [t, d] for all 8 heads at once (+bias via ones row)
                for ii in range(4):
                    i = 4 * j + ii
                    si = slice(ii * 128, (ii + 1) * 128)
                    ps = ps1.tile([128, 512], f32, tag="qkv", bufs=4)
                    for et in range(ET):
                        nc.tensor.matmul(
                            ps,
                            lhsT=xt_t[:, et, si],
                            rhs=wv_sb[:, et, :],
                            start=(et == 0),
                            stop=False,
                        )
                    nc.tensor.matmul(
                        ps,
                        lhsT=xt_t[0:1, 8, si],
                        rhs=wv_sb[0:1, 8, :],
                        start=False,
                        stop=True,
                    )
                    va_i = va[:, i, :].rearrange("p (h c) -> p h c", c=VW)
                    nc.vector.tensor_copy(
                        va_i[:, :, 0:64], ps.rearrange("p (h d) -> p h d", d=64)
                    )
                    if MM_DTYPE == "float32r":
                        nc.vector.memset(
                            va_i[:, :, 64:65].bitcast(mybir.dt.uint32),
                            0x3F800000,
                        )
                        nc.vector.memset(
                            va_i[:, :, 65:66].bitcast(mybir.dt.uint32), 0
                        )
                    else:
                        nc.vector.memset(va_i[:, :, 64:65], 1.0)
                        nc.vector.memset(va_i[:, :, 65:66], 0.0)

        # ---------------- Phase 2: causal attention ----------------
        with tc.tile_pool(name="ph2", bufs=1) as p2, tc.tile_pool(
            name="ps2", bufs=1, space="PSUM"
        ) as ps2:
            mask_sb = p2.tile([128, 4, 512], mdt, bufs=1)
            nc.sync.dma_start(out=mask_sb, in_=mask_d)
            for r in range(PAIRS):
                for j in range(SCH):
                    sj = slice(j * 512, (j + 1) * 512)
                    nt = 4 * j + 4  # causal: t-blocks 0 .. 4j+3
                    outps = [
                        ps2.tile([VW, 512], f32, tag=f"o{hh}", bufs=1,
                                 name=f"outp{hh}")
                        for hh in range(2)
                    ]
                    for ti in range(nt):
                        tis = slice(ti * 128, (ti + 1) * 128)
                        prs = []
                        for hh in range(2):
                            po = hh * 64
                            scp = ps2.tile(
                                [128, 512], f32, tag=f"sc{hh}", bufs=2
                            )
                            nc.tensor.matmul(
                                scp,
                                lhsT=kt[po : po + 64, r, tis],
                                rhs=qt[po : po + 64, r, sj],
                                start=True,
                                stop=True,
                            )
                            pr = p2.tile([128, 512], mdt, tag=f"pr{hh}", bufs=3)
                            nc.scalar.activation(pr, scp, AF.Exp, scale=SCALE)
                            if ti >= 4 * j:
                                nc.vector.tensor_mul(
                                    pr, pr, mask_sb[:, ti - 4 * j, :]
                                )
                            prs.append(pr)
                        for hh in range(2):
                            h = 2 * r + hh
                            nc.tensor.matmul(
                                outps[hh],
                                lhsT=va[:, ti, h * VW : (h + 1) * VW],
                                rhs=prs[hh],
                                start=(ti == 0),
                                stop=(ti == nt - 1),
                            )
                    # normalize and write concat^T to DRAM scratch
                    for hh in range(2):
                        po = hh * 64
                        dn = p2.tile([65, 512], mdt, tag="dn", bufs=2)
                        nc.vector.tensor_copy(dn[64:65, :], outps[hh][64:65, :])
                        bc = ps2.tile([64, 512], f32, tag="bc", bufs=2)
                        nc.tensor.matmul(
                            bc,
                            lhsT=ones65[64:65, :],
                            rhs=dn[64:65, :],
                            start=True,
                            stop=True,
                        )
                        rd = p2.tile([64, 512], f32, tag="rd", bufs=2)
                        rs = p2.tile([64, 512], f32, tag="rs", bufs=2)
                        nc.vector.reciprocal_approx_accurate(rd, bc, scratch=rs)
                        cw = p2.tile([64, 512], mdt, tag="cw", bufs=3)
                        nc.vector.tensor_mul(cw, outps[hh][0:64, :], rd)
                        nc.sync.dma_start(
                            out=cat_d[r, po : po + 64, sj], in_=cw
                        )

        # ---------------- Phase 3: output projection (partial) ----------------
        with tc.tile_pool(name="ph3", bufs=1) as p3, tc.tile_pool(
            name="ps3", bufs=1, space="PSUM"
        ) as ps3:
            wp_sb = p3.tile([128, PAIRS, E], mdt, bufs=1)
            nc.sync.dma_start(out=wp_sb, in_=wp_d)
            cat_view = cat_d.rearrange("r p s -> p r s")
            for sb in range(16):
                ss = slice(sb * 128, (sb + 1) * 128)
                ct = p3.tile([128, PAIRS, 128], mdt, tag="ct", bufs=3)
                nc.sync.dma_start(out=ct, in_=cat_view[:, :, ss])
                for f in range(2):
                    sf = slice(f * 512, (f + 1) * 512)
                    yp = ps3.tile([128, 512], f32, tag="y", bufs=4)
                    for r in range(PAIRS):
                        nc.tensor.matmul(
                            yp,
                            lhsT=ct[:, r, :],
                            rhs=wp_sb[:, r, sf],
                            start=(r == 0),
                            stop=(r == PAIRS - 1),
                        )
                    ys = p3.tile([128, 512], f32, tag="ys", bufs=3)
                    nc.vector.tensor_copy(ys, yp)
                    nc.sync.dma_start(out=y_d[ss, sf], in_=ys)

    nc.compile()
    return nc


def get_nc():
    if "nc" not in _CACHE:
        _CACHE["nc"] = _build()
    return _CACHE["nc"]


def prep_core_inputs(x, Wq, bq, Wk, bk, Wv, bv, Wp, core):
    """Pack the full-model inputs into one core's input map."""
    b, g = core // 2, core % 2
    heads = list(range(g * 8, g * 8 + 8))

    xa = np.zeros((9 * 128, S), np.float32)
    xa[:E] = x[b].T
    xa[E] = 1.0

    def pack_w(W):  # [H,E,D] -> local [E, 512] -> [128, 8, 512]
        Wl = np.concatenate([W[h] for h in heads], axis=1)
        return host_round(Wl.reshape(ET, 128, 512).transpose(1, 0, 2))

    wv_aug = np.zeros((9 * 128, 512), np.float32)
    wv_aug[:E] = np.concatenate([Wv[h] for h in heads], axis=1)
    wv_aug[E] = np.concatenate([bv[h] for h in heads])

    wp_l = host_round(
        Wp[g * 512 : (g + 1) * 512].reshape(PAIRS, 128, E).transpose(1, 0, 2)
    )

    def pack_b(bias):
        return np.stack(
            [
                np.concatenate([bias[heads[2 * r]], bias[heads[2 * r + 1]]])
                for r in range(PAIRS)
            ],
            axis=1,
        ).astype(np.float32)

    p = np.arange(128)[:, None, None]
    v = np.arange(4)[None, :, None]
    c = np.arange(512)[None, None, :]
    mask = host_round((c >= p + 128 * v).astype(np.float32))

    return {
        "xt": host_round(xa.reshape(9, 128, S)),
        "wq": pack_w(Wq),
        "wk": pack_w(Wk),
        "wv": host_round(wv_aug.reshape(9, 128, 512).transpose(1, 0, 2)),
        "wp": wp_l,
        "bq": pack_b(bq),
        "bk": pack_b(bk),
        "mask": mask,
    }


def kernel(**inputs):
    from concourse.bass_utils import run_bass_kernel_spmd

    args = {k: np.asarray(v, np.float32) for k, v in inputs.items()}
    nc = get_nc()
    in_maps = [
        prep_core_inputs(
            args["x"], args["Wq"], args["bq"], args["Wk"], args["bk"],
            args["Wv"], args["bv"], args["Wp"], c,
        )
        for c in range(NCORES)
    ]
    res = run_bass_kernel_spmd(nc, in_maps, core_ids=list(range(NCORES)))
    parts = [r["y"] for r in res.results]
    out = np.stack([parts[2 * b] + parts[2 * b + 1] for b in range(B)])
    return (out + args["bp"][None, None, :]).astype(np.float32)


# revision 10
# speedup vs baseline: 1.5803x; 1.2658x over previous
"""Multi-head causal attention (B=4, S=2048, E=1024, H=16, D=64) on 8 trn2 cores.

Sharding: core c handles batch b = c//2 and head-group g = c%2 (8 heads each).
Each core computes its partial output projection over its 512 local concat
columns; the host sums the two partials per batch and adds bp.

Layout strategy (per core):
  - x is pre-transposed on host: xT_aug [1025(+pad), S] with a ones row so the
    V bias rides along the matmul.
  - Q^T, K^T computed as [d, s] (d on partitions, 2 heads per 128-partition
    pair tile) so scores come out transposed: scoresT [t, s].
  - V kept natural [t, d] with a ones column appended per head (padded to 66
    cols for fp32r even-stride rules), so the PV matmul also produces the
    softmax denominator as row 64 of its output.
  - Softmax: exp on ACT (no max subtraction -- scores are O(1) by
    construction), causal masking via multiplicative 0/1 masks on DVE,
    denominator broadcast across partitions via a K=1 matmul, reciprocal on
    DVE, normalize into concat^T, output projection directly from concat^T.

MM_DTYPE picks the matmul input precision: float32 (exact, 4 cyc/row),
float32r (TF32, 1 cyc/row at N>=512), bfloat16 (1 cyc/row).
"""

import numpy as np

B, S, E, H, D = 4, 2048, 1024, 16, 64
NCORES = 8
PAIRS = 4  # head pairs per core (8 heads)
ET = 8  # e-tiles of 128 for the contraction over E
SCH = 4  # s-chunks of 512
VW = 66  # V columns per head: 64 d + 1 ones + 1 pad
SCALE = float(D) ** -0.5

MM_DTYPE = "bfloat16"

_CACHE = {}


def _round_tf32(a):
    b = np.ascontiguousarray(a, np.float32).view(np.uint32).copy()
    b += 0x0FFF + ((b >> 13) & 1)  # round-to-nearest-even into 19 bits
    b &= np.uint32(0xFFFFE000)
    return b.view(np.float32)


def host_round(a):
    a = np.ascontiguousarray(a, np.float32)
    if MM_DTYPE == "float32r":
        return _round_tf32(a)
    if MM_DTYPE == "bfloat16":
        import ml_dtypes

        return a.astype(ml_dtypes.bfloat16)
    return a


def _build():
    import concourse.tile as tile
    from concourse import bacc, mybir
    from contextlib import ExitStack

    f32 = mybir.dt.float32
    mdt = getattr(mybir.dt, MM_DTYPE)
    AF = mybir.ActivationFunctionType

    nc = bacc.Bacc("TRN2", target_bir_lowering=False, debug=False, num_devices=NCORES)

    xt_d = nc.dram_tensor("xt", [9, 128, S], mdt, kind="ExternalInput").ap()
    wq_d = nc.dram_tensor("wq", [128, ET, 512], mdt, kind="ExternalInput").ap()
    wk_d = nc.dram_tensor("wk", [128, ET, 512], mdt, kind="ExternalInput").ap()
    wv_d = nc.dram_tensor("wv", [128, 9, 512], mdt, kind="ExternalInput").ap()
    wp_d = nc.dram_tensor("wp", [128, PAIRS, E], mdt, kind="ExternalInput").ap()
    bq_d = nc.dram_tensor("bq", [128, PAIRS], f32, kind="ExternalInput").ap()
    bk_d = nc.dram_tensor("bk", [128, PAIRS], f32, kind="ExternalInput").ap()
    mask_d = nc.dram_tensor("mask", [128, 4, 512], mdt, kind="ExternalInput").ap()
    y_d = nc.dram_tensor("y", [S, E], f32, kind="ExternalOutput").ap()
    cat_d = nc.dram_tensor("cat_scratch", [PAIRS, 128, S], mdt).ap()

    with tile.TileContext(nc) as tc, ExitStack() as ctx:
        pers = ctx.enter_context(tc.tile_pool(name="pers", bufs=1))
        qt = pers.tile([128, PAIRS, S], mdt)  # Q^T pair tiles
        kt = pers.tile([128, PAIRS, S], mdt)  # K^T pair tiles
        va = pers.tile([128, 16, 8 * VW], mdt)  # V (+ones col) per t-block
        bq_sb = pers.tile([128, PAIRS], f32)
        bk_sb = pers.tile([128, PAIRS], f32)
        ones65 = pers.tile([65, 64], mdt)
        nc.sync.dma_start(out=bq_sb, in_=bq_d)
        nc.sync.dma_start(out=bk_sb, in_=bk_d)
        if MM_DTYPE == "float32r":
            nc.vector.memset(ones65.bitcast(mybir.dt.uint32), 0x3F800000)
        else:
            nc.vector.memset(ones65, 1.0)

        # ---------------- Phase 1: QKV projections ----------------
        with tc.tile_pool(name="ph1", bufs=1) as p1, tc.tile_pool(
            name="ps1", bufs=1, space="PSUM"
        ) as ps1:
            wq_sb = p1.tile([128, ET, 512], mdt, bufs=1)
            wk_sb = p1.tile([128, ET, 512], mdt, bufs=1)
            wv_sb = p1.tile([128, 9, 512], mdt, bufs=1)
            nc.sync.dma_start(out=wq_sb, in_=wq_d)
            nc.sync.dma_start(out=wk_sb, in_=wk_d)
            nc.sync.dma_start(out=wv_sb, in_=wv_d)
            xt_view = xt_d.rearrange("e p s -> p e s")
            for j in range(SCH):
                sj = slice(j * 512, (j + 1) * 512)
                xt_t = p1.tile([128, 9, 512], mdt, tag="xt", bufs=2)
                nc.sync.dma_start(out=xt_t, in_=xt_view[:, :, sj])
                # Q^T and K^T: [d_pair, s] per pair
                for r in range(PAIRS):
                    for w_sb, dst, b_sb in (
                        (wq_sb, qt, bq_sb),
                        (wk_sb, kt, bk_sb),
                    ):
                        ps = ps1.tile([128, 512], f32, tag="qkv", bufs=4)
                        for et in range(ET):
                            nc.tensor.matmul(
                                ps,
                                lhsT=w_sb[:, et, r * 128 : (r + 1) * 128],
                                rhs=xt_t[:, et, :],
                                start=(et == 0),
                                stop=(et == ET - 1),
                            )
                        nc.vector.tensor_scalar_add(
                            dst[:, r, sj], ps, b_sb[:, r : r + 1]
                        )
                # V natural [t, d] for all 8 heads at once (+bias via ones row)
                for ii in range(4):
                    i = 4 * j + ii
                    si = slice(ii * 128, (ii + 1) * 128)
                    ps = ps1.tile([128, 512], f32, tag="qkv", bufs=4)
                    for et in range(ET):
                        nc.tensor.matmul(
                            ps,
                            lhsT=xt_t[:, et, si],
                            rhs=wv_sb[:, et, :],
                            start=(et == 0),
                            stop=False,
                        )
                    nc.tensor.matmul(
                        ps,
                        lhsT=xt_t[0:1, 8, si],
                        rhs=wv_sb[0:1, 8, :],
                        start=False,
                        stop=True,
                    )
                    va_i = va[:, i, :].rearrange("p (h c) -> p h c", c=VW)
                    nc.vector.tensor_copy(
                        va_i[:, :, 0:64], ps.rearrange("p (h d) -> p h d", d=64)
                    )
                    if MM_DTYPE == "float32r":
                        nc.vector.memset(
                            va_i[:, :, 64:65].bitcast(mybir.dt.uint32),
                            0x3F800000,
                        )
                        nc.vector.memset(
                            va_i[:, :, 65:66].bitcast(mybir.dt.uint32), 0
                        )
                    else:
                        nc.vector.memset(va_i[:, :, 64:65], 1.0)
                        nc.vector.memset(va_i[:, :, 65:66], 0.0)

        # ---------------- Phase 2: causal attention ----------------
        with tc.tile_pool(name="ph2", bufs=1) as p2, tc.tile_pool(
            name="ps2", bufs=1, space="PSUM"
        ) as ps2:
            mask_sb = p2.tile([128, 4, 512], mdt, bufs=1)
            nc.sync.dma_start(out=mask_sb, in_=mask_d)
            for r in range(PAIRS):
                for j in range(SCH):
                    sj = slice(j * 512, (j + 1) * 512)
                    nt = 4 * j + 4  # causal: t-blocks 0 .. 4j+3
                    outps = [
                        ps2.tile([VW, 512], f32, tag=f"o{hh}", bufs=1,
                                 name=f"outp{hh}")
                        for hh in range(2)
                    ]
                    for ti in range(nt):
                        tis = slice(ti * 128, (ti + 1) * 128)
                        # causal column offset within this s-chunk: the
                        # diagonal t-blocks only reach s-columns >= 128v
                        v = max(ti - 4 * j, 0)
                        w = 512 - 128 * v
                        sjv = slice(j * 512 + 128 * v, (j + 1) * 512)
                        # both heads' scores side by side in one 2-bank psum
                        scp = ps2.tile([128, 2, 512], f32, tag="sc", bufs=2)
                        for hh in range(2):
                            po = hh * 64
                            nc.tensor.matmul(
                                scp[:, hh, 128 * v :],
                                lhsT=kt[po : po + 64, r, tis],
                                rhs=qt[po : po + 64, r, sjv],
                                start=True,
                                stop=True,
                            )
                        pr = p2.tile([128, 2, 512], mdt, tag="pr", bufs=3)
                        nc.scalar.activation(
                            pr[:, :, 128 * v :],
                            scp[:, :, 128 * v :],
                            AF.Exp,
                            scale=SCALE,
                        )
                        if v or ti == 4 * j:
                            for hh in range(2):
                                nc.vector.tensor_mul(
                                    pr[:, hh, 128 * v :],
                                    pr[:, hh, 128 * v :],
                                    mask_sb[:, v, 128 * v :],
                                )
                        for hh in range(2):
                            h = 2 * r + hh
                            nc.tensor.matmul(
                                outps[hh][:, 128 * v :],
                                lhsT=va[:, ti, h * VW : (h + 1) * VW],
                                rhs=pr[:, hh, 128 * v :],
                                start=(ti == 0),
                                stop=(ti == nt - 1),
                            )
                    # normalize and write concat^T to DRAM scratch
                    for hh in range(2):
                        po = hh * 64
                        dn = p2.tile([65, 512], mdt, tag="dn", bufs=2)
                        nc.vector.tensor_copy(dn[64:65, :], outps[hh][64:65, :])
                        bc = ps2.tile([64, 512], f32, tag="bc", bufs=2)
                        nc.tensor.matmul(
                            bc,
                            lhsT=ones65[64:65, :],
                            rhs=dn[64:65, :],
                            start=True,
                            stop=True,
                        )
                        rd = p2.tile([64, 512], f32, tag="rd", bufs=2)
                        nc.vector.reciprocal_approx_fast(rd, bc)
                        cw = p2.tile([64, 512], mdt, tag="cw", bufs=3)
                        nc.vector.tensor_mul(cw, outps[hh][0:64, :], rd)
                        nc.sync.dma_start(
                            out=cat_d[r, po : po + 64, sj], in_=cw
                        )

        # ---------------- Phase 3: output projection (partial) ----------------
        with tc.tile_pool(name="ph3", bufs=1) as p3, tc.tile_pool(
            name="ps3", bufs=1, space="PSUM"
        ) as ps3:
            wp_sb = p3.tile([128, PAIRS, E], mdt, bufs=1)
            nc.sync.dma_start(out=wp_sb, in_=wp_d)
            cat_view = cat_d.rearrange("r p s -> p r s")
            for sb in range(16):
                ss = slice(sb * 128, (sb + 1) * 128)
                ct = p3.tile([128, PAIRS, 128], mdt, tag="ct", bufs=3)
                nc.sync.dma_start(out=ct, in_=cat_view[:, :, ss])
                for f in range(2):
                    sf = slice(f * 512, (f + 1) * 512)
                    yp = ps3.tile([128, 512], f32, tag="y", bufs=4)
                    for r in range(PAIRS):
                        nc.tensor.matmul(
                            yp,
                            lhsT=ct[:, r, :],
                            rhs=wp_sb[:, r, sf],
                            start=(r == 0),
                            stop=(r == PAIRS - 1),
                        )
                    ys = p3.tile([128, 512], f32, tag="ys", bufs=3)
                    nc.vector.tensor_copy(ys, yp)
                    nc.sync.dma_start(out=y_d[ss, sf], in_=ys)

    nc.compile()
    return nc


def get_nc():
    if "nc" not in _CACHE:
        _CACHE["nc"] = _build()
    return _CACHE["nc"]


def prep_core_inputs(x, Wq, bq, Wk, bk, Wv, bv, Wp, core):
    """Pack the full-model inputs into one core's input map."""
    b, g = core // 2, core % 2
    heads = list(range(g * 8, g * 8 + 8))

    xa = np.zeros((9 * 128, S), np.float32)
    xa[:E] = x[b].T
    xa[E] = 1.0

    def pack_w(W):  # [H,E,D] -> local [E, 512] -> [128, 8, 512]
        Wl = np.concatenate([W[h] for h in heads], axis=1)
        return host_round(Wl.reshape(ET, 128, 512).transpose(1, 0, 2))

    wv_aug = np.zeros((9 * 128, 512), np.float32)
    wv_aug[:E] = np.concatenate([Wv[h] for h in heads], axis=1)
    wv_aug[E] = np.concatenate([bv[h] for h in heads])

    wp_l = host_round(
        Wp[g * 512 : (g + 1) * 512].reshape(PAIRS, 128, E).transpose(1, 0, 2)
    )

    def pack_b(bias):
        return np.stack(
            [
                np.concatenate([bias[heads[2 * r]], bias[heads[2 * r + 1]]])
                for r in range(PAIRS)
            ],
            axis=1,
        ).astype(np.float32)

    p = np.arange(128)[:, None, None]
    v = np.arange(4)[None, :, None]
    c = np.arange(512)[None, None, :]
    mask = host_round((c >= p + 128 * v).astype(np.float32))

    return {
        "xt": host_round(xa.reshape(9, 128, S)),
        "wq": pack_w(Wq),
        "wk": pack_w(Wk),
        "wv": host_round(wv_aug.reshape(9, 128, 512).transpose(1, 0, 2)),
        "wp": wp_l,
        "bq": pack_b(bq),
        "bk": pack_b(bk),
        "mask": mask,
    }


def kernel(**inputs):
    from concourse.bass_utils import run_bass_kernel_spmd

    args = {k: np.asarray(v, np.float32) for k, v in inputs.items()}
    nc = get_nc()
    in_maps = [
        prep_core_inputs(
            args["x"], args["Wq"], args["bq"], args["Wk"], args["bk"],
            args["Wv"], args["bv"], args["Wp"], c,
        )
        for c in range(NCORES)
    ]
    res = run_bass_kernel_spmd(nc, in_maps, core_ids=list(range(NCORES)))
    parts = [r["y"] for r in res.results]
    out = np.stack([parts[2 * b] + parts[2 * b + 1] for b in range(B)])
    return (out + args["bp"][None, None, :]).astype(np.float32)


# revision 11
# speedup vs baseline: 1.6993x; 1.0753x over previous
"""Multi-head causal attention (B=4, S=2048, E=1024, H=16, D=64) on 8 trn2 cores.

Sharding: core c handles batch b = c//2 and head-group g = c%2 (8 heads each).
Each core computes its partial output projection over its 512 local concat
columns; the host sums the two partials per batch and adds bp.

Layout strategy (per core):
  - x is pre-transposed on host: xT_aug [1025(+pad), S] with a ones row so the
    V bias rides along the matmul.
  - Q^T, K^T computed as [d, s] (d on partitions, 2 heads per 128-partition
    pair tile) so scores come out transposed: scoresT [t, s].
  - V kept natural [t, d] with a ones column appended per head (padded to 66
    cols for fp32r even-stride rules), so the PV matmul also produces the
    softmax denominator as row 64 of its output.
  - Softmax: exp on ACT (no max subtraction -- scores are O(1) by
    construction), causal masking via multiplicative 0/1 masks on DVE,
    denominator broadcast across partitions via a K=1 matmul, reciprocal on
    DVE, normalize into concat^T, output projection directly from concat^T.

MM_DTYPE picks the matmul input precision: float32 (exact, 4 cyc/row),
float32r (TF32, 1 cyc/row at N>=512), bfloat16 (1 cyc/row).
"""

import numpy as np

B, S, E, H, D = 4, 2048, 1024, 16, 64
NCORES = 8
PAIRS = 4  # head pairs per core (8 heads)
ET = 8  # e-tiles of 128 for the contraction over E
SCH = 4  # s-chunks of 512
VW = 66  # V columns per head: 64 d + 1 ones + 1 pad
SCALE = float(D) ** -0.5

MM_DTYPE = "bfloat16"

_CACHE = {}


def _round_tf32(a):
    b = np.ascontiguousarray(a, np.float32).view(np.uint32).copy()
    b += 0x0FFF + ((b >> 13) & 1)  # round-to-nearest-even into 19 bits
    b &= np.uint32(0xFFFFE000)
    return b.view(np.float32)


def host_round(a):
    a = np.ascontiguousarray(a, np.float32)
    if MM_DTYPE == "float32r":
        return _round_tf32(a)
    if MM_DTYPE == "bfloat16":
        import ml_dtypes

        return a.astype(ml_dtypes.bfloat16)
    return a


def _build():
    import concourse.tile as tile
    from concourse import bacc, mybir
    from contextlib import ExitStack

    f32 = mybir.dt.float32
    mdt = getattr(mybir.dt, MM_DTYPE)
    AF = mybir.ActivationFunctionType

    nc = bacc.Bacc("TRN2", target_bir_lowering=False, debug=False, num_devices=NCORES)

    xt_d = nc.dram_tensor("xt", [9, 128, S], mdt, kind="ExternalInput").ap()
    wq_d = nc.dram_tensor("wq", [128, ET, 512], mdt, kind="ExternalInput").ap()
    wk_d = nc.dram_tensor("wk", [128, ET, 512], mdt, kind="ExternalInput").ap()
    wv_d = nc.dram_tensor("wv", [128, 9, 512], mdt, kind="ExternalInput").ap()
    wp_d = nc.dram_tensor("wp", [128, PAIRS, E], mdt, kind="ExternalInput").ap()
    bq_d = nc.dram_tensor("bq", [128, PAIRS], f32, kind="ExternalInput").ap()
    bk_d = nc.dram_tensor("bk", [128, PAIRS], f32, kind="ExternalInput").ap()
    mask_d = nc.dram_tensor("mask", [128, 4, 512], mdt, kind="ExternalInput").ap()
    y_d = nc.dram_tensor("y", [S, E], f32, kind="ExternalOutput").ap()

    with tile.TileContext(nc) as tc, ExitStack() as ctx:
        pers = ctx.enter_context(tc.tile_pool(name="pers", bufs=1))
        work = ctx.enter_context(tc.tile_pool(name="work", bufs=1))
        psp = ctx.enter_context(tc.tile_pool(name="psp", bufs=1, space="PSUM"))

        qt = pers.tile([128, PAIRS, S], mdt)  # Q^T pair tiles
        kt = pers.tile([128, PAIRS, S], mdt)  # K^T pair tiles
        va = pers.tile([128, 16, 8 * VW], mdt)  # V (+ones col) per t-block
        cat = pers.tile([128, PAIRS, S], mdt)  # concat^T
        bq_sb = pers.tile([128, PAIRS], f32)
        bk_sb = pers.tile([128, PAIRS], f32)
        ones65 = pers.tile([65, 64], mdt)
        wq_sb = pers.tile([128, ET, 512], mdt)
        wk_sb = pers.tile([128, ET, 512], mdt)
        wv_sb = pers.tile([128, 9, 512], mdt)
        wp_sb = pers.tile([128, PAIRS, E], mdt)
        mask_sb = pers.tile([128, 4, 512], mdt)
        nc.sync.dma_start(out=bq_sb, in_=bq_d)
        nc.sync.dma_start(out=bk_sb, in_=bk_d)
        nc.sync.dma_start(out=wq_sb, in_=wq_d)
        nc.sync.dma_start(out=wk_sb, in_=wk_d)
        nc.sync.dma_start(out=wv_sb, in_=wv_d)
        nc.sync.dma_start(out=wp_sb, in_=wp_d)
        nc.sync.dma_start(out=mask_sb, in_=mask_d)
        if MM_DTYPE == "float32r":
            nc.vector.memset(ones65.bitcast(mybir.dt.uint32), 0x3F800000)
        else:
            nc.vector.memset(ones65, 1.0)

        xt_view = xt_d.rearrange("e p s -> p e s")
        for j in range(SCH):
            sj = slice(j * 512, (j + 1) * 512)
            # ---- QKV projections for s-chunk j ----
            xt_t = work.tile([128, 9, 512], mdt, tag="xt", bufs=2)
            nc.sync.dma_start(out=xt_t, in_=xt_view[:, :, sj])
            for r in range(PAIRS):
                for w_sb, dst, b_sb in (
                    (wq_sb, qt, bq_sb),
                    (wk_sb, kt, bk_sb),
                ):
                    ps = psp.tile([128, 512], f32, tag="mm512", bufs=2)
                    for et in range(ET):
                        nc.tensor.matmul(
                            ps,
                            lhsT=w_sb[:, et, r * 128 : (r + 1) * 128],
                            rhs=xt_t[:, et, :],
                            start=(et == 0),
                            stop=(et == ET - 1),
                        )
                    nc.vector.tensor_scalar_add(
                        dst[:, r, sj], ps, b_sb[:, r : r + 1]
                    )
            for ii in range(4):
                i = 4 * j + ii
                si = slice(ii * 128, (ii + 1) * 128)
                ps = psp.tile([128, 512], f32, tag="mm512", bufs=2)
                for et in range(ET):
                    nc.tensor.matmul(
                        ps,
                        lhsT=xt_t[:, et, si],
                        rhs=wv_sb[:, et, :],
                        start=(et == 0),
                        stop=False,
                    )
                nc.tensor.matmul(
                    ps,
                    lhsT=xt_t[0:1, 8, si],
                    rhs=wv_sb[0:1, 8, :],
                    start=False,
                    stop=True,
                )
                va_i = va[:, i, :].rearrange("p (h c) -> p h c", c=VW)
                nc.vector.tensor_copy(
                    va_i[:, :, 0:64], ps.rearrange("p (h d) -> p h d", d=64)
                )
                if MM_DTYPE == "float32r":
                    nc.vector.memset(
                        va_i[:, :, 64:65].bitcast(mybir.dt.uint32), 0x3F800000
                    )
                    nc.vector.memset(
                        va_i[:, :, 65:66].bitcast(mybir.dt.uint32), 0
                    )
                else:
                    nc.vector.memset(va_i[:, :, 64:65], 1.0)
                    nc.vector.memset(va_i[:, :, 65:66], 0.0)

            # ---- causal attention for s-chunk j, all 4 head pairs ----
            for r in range(PAIRS):
                nt = 4 * j + 4  # causal: t-blocks 0 .. 4j+3
                outps = [
                    psp.tile([VW, 512], f32, tag=f"o{hh}", bufs=1,
                             name=f"outp{hh}")
                    for hh in range(2)
                ]
                for ti in range(nt):
                    tis = slice(ti * 128, (ti + 1) * 128)
                    v = max(ti - 4 * j, 0)
                    sjv = slice(j * 512 + 128 * v, (j + 1) * 512)
                    scp = psp.tile([128, 2, 512], f32, tag="sc", bufs=2)
                    for hh in range(2):
                        po = hh * 64
                        nc.tensor.matmul(
                            scp[:, hh, 128 * v :],
                            lhsT=kt[po : po + 64, r, tis],
                            rhs=qt[po : po + 64, r, sjv],
                            start=True,
                            stop=True,
                        )
                    pr = work.tile([128, 2, 512], mdt, tag="pr", bufs=4)
                    nc.scalar.activation(
                        pr[:, :, 128 * v :],
                        scp[:, :, 128 * v :],
                        AF.Exp,
                        scale=SCALE,
                    )
                    if v or ti == 4 * j:
                        for hh in range(2):
                            nc.vector.tensor_mul(
                                pr[:, hh, 128 * v :],
                                pr[:, hh, 128 * v :],
                                mask_sb[:, v, 128 * v :],
                            )
                    for hh in range(2):
                        h = 2 * r + hh
                        nc.tensor.matmul(
                            outps[hh][:, 128 * v :],
                            lhsT=va[:, ti, h * VW : (h + 1) * VW],
                            rhs=pr[:, hh, 128 * v :],
                            start=(ti == 0),
                            stop=(ti == nt - 1),
                        )
                # normalize into concat^T (in SBUF)
                for hh in range(2):
                    po = hh * 64
                    dn = work.tile([65, 512], mdt, tag="dn", bufs=2)
                    nc.vector.tensor_copy(dn[64:65, :], outps[hh][64:65, :])
                    bc = psp.tile([64, 512], f32, tag="mm512", bufs=2,
                                  name="bcst")
                    nc.tensor.matmul(
                        bc,
                        lhsT=ones65[64:65, :],
                        rhs=dn[64:65, :],
                        start=True,
                        stop=True,
                    )
                    rd = work.tile([64, 512], f32, tag="rd", bufs=2)
                    nc.vector.reciprocal_approx_fast(rd, bc)
                    nc.vector.tensor_mul(
                        cat[po : po + 64, r, sj], outps[hh][0:64, :], rd
                    )

            # ---- output projection for s-chunk j ----
            for sb in range(4 * j, 4 * j + 4):
                ss = slice(sb * 128, (sb + 1) * 128)
                for f in range(2):
                    sf = slice(f * 512, (f + 1) * 512)
                    yp = psp.tile([128, 512], f32, tag="mm512", bufs=2,
                                  name="yproj")
                    for r in range(PAIRS):
                        nc.tensor.matmul(
                            yp,
                            lhsT=cat[:, r, ss],
                            rhs=wp_sb[:, r, sf],
                            start=(r == 0),
                            stop=(r == PAIRS - 1),
                        )
                    ys = work.tile([128, 512], f32, tag="ys", bufs=3)
                    nc.vector.tensor_copy(ys, yp)
                    nc.sync.dma_start(out=y_d[ss, sf], in_=ys)

    nc.compile()
    return nc


def get_nc():
    if "nc" not in _CACHE:
        _CACHE["nc"] = _build()
    return _CACHE["nc"]


def prep_core_inputs(x, Wq, bq, Wk, bk, Wv, bv, Wp, core):
    """Pack the full-model inputs into one core's input map."""
    b, g = core // 2, core % 2
    heads = list(range(g * 8, g * 8 + 8))

    xa = np.zeros((9 * 128, S), np.float32)
    xa[:E] = x[b].T
    xa[E] = 1.0

    def pack_w(W):  # [H,E,D] -> local [E, 512] -> [128, 8, 512]
        Wl = np.concatenate([W[h] for h in heads], axis=1)
        return host_round(Wl.reshape(ET, 128, 512).transpose(1, 0, 2))

    wv_aug = np.zeros((9 * 128, 512), np.float32)
    wv_aug[:E] = np.concatenate([Wv[h] for h in heads], axis=1)
    wv_aug[E] = np.concatenate([bv[h] for h in heads])

    wp_l = host_round(
        Wp[g * 512 : (g + 1) * 512].reshape(PAIRS, 128, E).transpose(1, 0, 2)
    )

    def pack_b(bias):
        return np.stack(
            [
                np.concatenate([bias[heads[2 * r]], bias[heads[2 * r + 1]]])
                for r in range(PAIRS)
            ],
            axis=1,
        ).astype(np.float32)

    p = np.arange(128)[:, None, None]
    v = np.arange(4)[None, :, None]
    c = np.arange(512)[None, None, :]
    mask = host_round((c >= p + 128 * v).astype(np.float32))

    return {
        "xt": host_round(xa.reshape(9, 128, S)),
        "wq": pack_w(Wq),
        "wk": pack_w(Wk),
        "wv": host_round(wv_aug.reshape(9, 128, 512).transpose(1, 0, 2)),
        "wp": wp_l,
        "bq": pack_b(bq),
        "bk": pack_b(bk),
        "mask": mask,
    }


def kernel(**inputs):
    from concourse.bass_utils import run_bass_kernel_spmd

    args = {k: np.asarray(v, np.float32) for k, v in inputs.items()}
    nc = get_nc()
    in_maps = [
        prep_core_inputs(
            args["x"], args["Wq"], args["bq"], args["Wk"], args["bk"],
            args["Wv"], args["bv"], args["Wp"], c,
        )
        for c in range(NCORES)
    ]
    res = run_bass_kernel_spmd(nc, in_maps, core_ids=list(range(NCORES)))
    parts = [r["y"] for r in res.results]
    out = np.stack([parts[2 * b] + parts[2 * b + 1] for b in range(B)])
    return (out + args["bp"][None, None, :]).astype(np.float32)


# revision 13
# speedup vs baseline: 1.8933x; 1.1142x over previous
"""Multi-head causal attention (B=4, S=2048, E=1024, H=16, D=64) on 8 trn2 cores.

Sharding: core c handles batch b = c//2 and head-group g = c%2 (8 heads each).
Each core computes its partial output projection over its 512 local concat
columns; the host sums the two partials per batch and adds bp.

Layout strategy (per core):
  - x is pre-transposed on host: xT_aug [1025(+pad), S] with a ones row so the
    V bias rides along the matmul.
  - Q^T, K^T computed as [d, s] (d on partitions, 2 heads per 128-partition
    pair tile) so scores come out transposed: scoresT [t, s].
  - V kept natural [t, d] with a ones column appended per head (padded to 66
    cols for fp32r even-stride rules), so the PV matmul also produces the
    softmax denominator as row 64 of its output.
  - Softmax: exp on ACT (no max subtraction -- scores are O(1) by
    construction), causal masking via multiplicative 0/1 masks on DVE,
    denominator broadcast across partitions via a K=1 matmul, reciprocal on
    DVE, normalize into concat^T, output projection directly from concat^T.

MM_DTYPE picks the matmul input precision: float32 (exact, 4 cyc/row),
float32r (TF32, 1 cyc/row at N>=512), bfloat16 (1 cyc/row).
"""

import numpy as np

B, S, E, H, D = 4, 2048, 1024, 16, 64
NCORES = 8
PAIRS = 4  # head pairs per core (8 heads)
ET = 8  # e-tiles of 128 for the contraction over E
SCH = 4  # s-chunks of 512
VW = 66  # V columns per head: 64 d + 1 ones + 1 pad
SCALE = float(D) ** -0.5

MM_DTYPE = "bfloat16"

_CACHE = {}


def _round_tf32(a):
    b = np.ascontiguousarray(a, np.float32).view(np.uint32).copy()
    b += 0x0FFF + ((b >> 13) & 1)  # round-to-nearest-even into 19 bits
    b &= np.uint32(0xFFFFE000)
    return b.view(np.float32)


def host_round(a):
    a = np.ascontiguousarray(a, np.float32)
    if MM_DTYPE == "float32r":
        return _round_tf32(a)
    if MM_DTYPE == "bfloat16":
        import ml_dtypes

        return a.astype(ml_dtypes.bfloat16)
    return a


def _build():
    import concourse.tile as tile
    from concourse import bacc, mybir
    from contextlib import ExitStack

    f32 = mybir.dt.float32
    mdt = getattr(mybir.dt, MM_DTYPE)
    AF = mybir.ActivationFunctionType

    nc = bacc.Bacc("TRN2", target_bir_lowering=False, debug=False, num_devices=NCORES)

    xt_d = nc.dram_tensor("xt", [9, 128, S], mdt, kind="ExternalInput").ap()
    wq_d = nc.dram_tensor("wq", [128, ET, 512], mdt, kind="ExternalInput").ap()
    wk_d = nc.dram_tensor("wk", [128, ET, 512], mdt, kind="ExternalInput").ap()
    wv_d = nc.dram_tensor("wv", [128, 9, 512], mdt, kind="ExternalInput").ap()
    wp_d = nc.dram_tensor("wp", [128, PAIRS, E], mdt, kind="ExternalInput").ap()
    bq_d = nc.dram_tensor("bq", [128, PAIRS], f32, kind="ExternalInput").ap()
    bk_d = nc.dram_tensor("bk", [128, PAIRS], f32, kind="ExternalInput").ap()
    mask_d = nc.dram_tensor("mask", [128, 4, 512], mdt, kind="ExternalInput").ap()
    y_d = nc.dram_tensor("y", [S, E], f32, kind="ExternalOutput").ap()

    with tile.TileContext(nc) as tc, ExitStack() as ctx:
        pers = ctx.enter_context(tc.tile_pool(name="pers", bufs=1))
        work = ctx.enter_context(tc.tile_pool(name="work", bufs=1))
        psp = ctx.enter_context(tc.tile_pool(name="psp", bufs=1, space="PSUM"))

        qt = pers.tile([128, PAIRS, S], mdt)  # Q^T pair tiles
        kt = pers.tile([128, PAIRS, S], mdt)  # K^T pair tiles
        va = pers.tile([128, 16, 8 * VW], mdt)  # V (+ones col) per t-block
        cat = pers.tile([128, PAIRS, S], mdt)  # concat^T
        bq_sb = pers.tile([128, PAIRS], f32)
        bk_sb = pers.tile([128, PAIRS], f32)
        ones65 = pers.tile([65, 64], mdt)
        wq_sb = pers.tile([128, ET, 512], mdt)
        wk_sb = pers.tile([128, ET, 512], mdt)
        wv_sb = pers.tile([128, 9, 512], mdt)
        wp_sb = pers.tile([128, PAIRS, E], mdt)
        mask_sb = pers.tile([128, 4, 512], mdt)
        xts = [
            work.tile([128, 9, 512], mdt, tag=f"xt{j % 2}", name=f"xt{j}")
            for j in range(SCH)
        ]
        xt_view = xt_d.rearrange("e p s -> p e s")
        # DMA priority order: what phase-0 compute needs comes first
        nc.sync.dma_start(out=wq_sb, in_=wq_d)
        nc.sync.dma_start(out=xts[0], in_=xt_view[:, :, 0:512])
        nc.sync.dma_start(out=wk_sb, in_=wk_d)
        nc.sync.dma_start(out=wv_sb, in_=wv_d)
        nc.sync.dma_start(out=bq_sb, in_=bq_d)
        nc.sync.dma_start(out=bk_sb, in_=bk_d)
        nc.sync.dma_start(out=mask_sb, in_=mask_d)
        if MM_DTYPE == "float32r":
            nc.vector.memset(ones65.bitcast(mybir.dt.uint32), 0x3F800000)
        else:
            nc.vector.memset(ones65, 1.0)
        for j in range(1, SCH):
            nc.sync.dma_start(
                out=xts[j], in_=xt_view[:, :, j * 512 : (j + 1) * 512]
            )
        nc.sync.dma_start(out=wp_sb, in_=wp_d)

        # ---------------- work-item emitters ----------------
        def emit_qk(j, r, which):
            w_sb, dst, b_sb = (
                (wq_sb, qt, bq_sb) if which == "q" else (wk_sb, kt, bk_sb)
            )
            sjl = slice(j * 512, (j + 1) * 512)
            ps = psp.tile([128, 512], f32, tag="mm512", bufs=2, name="qkps")
            for et in range(ET):
                nc.tensor.matmul(
                    ps,
                    lhsT=w_sb[:, et, r * 128 : (r + 1) * 128],
                    rhs=xts[j][:, et, :],
                    start=(et == 0),
                    stop=(et == ET - 1),
                )
            nc.vector.tensor_scalar_add(dst[:, r, sjl], ps, b_sb[:, r : r + 1])

        def emit_v(j, ii):
            i = 4 * j + ii
            si = slice(ii * 128, (ii + 1) * 128)
            ps = psp.tile([128, 512], f32, tag="mm512", bufs=2, name="vps")
            for et in range(ET):
                nc.tensor.matmul(
                    ps,
                    lhsT=xts[j][:, et, si],
                    rhs=wv_sb[:, et, :],
                    start=(et == 0),
                    stop=False,
                )
            nc.tensor.matmul(
                ps,
                lhsT=xts[j][0:1, 8, si],
                rhs=wv_sb[0:1, 8, :],
                start=False,
                stop=True,
            )
            va_i = va[:, i, :].rearrange("p (h c) -> p h c", c=VW)
            nc.vector.tensor_copy(
                va_i[:, :, 0:64], ps.rearrange("p (h d) -> p h d", d=64)
            )
            if MM_DTYPE == "float32r":
                nc.vector.memset(
                    va_i[:, :, 64:65].bitcast(mybir.dt.uint32), 0x3F800000
                )
                nc.vector.memset(va_i[:, :, 65:66].bitcast(mybir.dt.uint32), 0)
            else:
                nc.vector.memset(va_i[:, :, 64:65], 1.0)
                nc.vector.memset(va_i[:, :, 65:66], 0.0)

        attn_state = {}

        def emit_attn_ti(j, r, ti):
            nt = 4 * j + 4
            if ti == 0:
                attn_state[(j, r)] = [
                    psp.tile([VW, 512], f32, tag=f"o{hh}", bufs=1,
                             name=f"outp{hh}")
                    for hh in range(2)
                ]
            outps = attn_state[(j, r)]
            tis = slice(ti * 128, (ti + 1) * 128)
            v = max(ti - 4 * j, 0)
            sjv = slice(j * 512 + 128 * v, (j + 1) * 512)
            scp = psp.tile([128, 2, 512], f32, tag="sc", bufs=2)
            for hh in range(2):
                po = hh * 64
                nc.tensor.matmul(
                    scp[:, hh, 128 * v :],
                    lhsT=kt[po : po + 64, r, tis],
                    rhs=qt[po : po + 64, r, sjv],
                    start=True,
                    stop=True,
                )
            pr = work.tile([128, 2, 512], mdt, tag="pr", bufs=4)
            nc.scalar.activation(
                pr[:, :, 128 * v :], scp[:, :, 128 * v :], AF.Exp, scale=SCALE
            )
            if v or ti == 4 * j:
                for hh in range(2):
                    nc.vector.tensor_mul(
                        pr[:, hh, 128 * v :],
                        pr[:, hh, 128 * v :],
                        mask_sb[:, v, 128 * v :],
                    )
            for hh in range(2):
                h = 2 * r + hh
                nc.tensor.matmul(
                    outps[hh][:, 128 * v :],
                    lhsT=va[:, ti, h * VW : (h + 1) * VW],
                    rhs=pr[:, hh, 128 * v :],
                    start=(ti == 0),
                    stop=(ti == nt - 1),
                )

        def emit_norm(j, r):
            outps = attn_state.pop((j, r))
            sjl = slice(j * 512, (j + 1) * 512)
            for hh in range(2):
                po = hh * 64
                dn = work.tile([65, 512], mdt, tag="dn", bufs=2)
                nc.vector.tensor_copy(dn[64:65, :], outps[hh][64:65, :])
                bc = psp.tile([64, 512], f32, tag="mm512", bufs=2, name="bcst")
                nc.tensor.matmul(
                    bc,
                    lhsT=ones65[64:65, :],
                    rhs=dn[64:65, :],
                    start=True,
                    stop=True,
                )
                rd = work.tile([64, 512], f32, tag="rd", bufs=2)
                nc.vector.reciprocal_approx_fast(rd, bc)
                nc.vector.tensor_mul(
                    cat[po : po + 64, r, sjl], outps[hh][0:64, :], rd
                )

        def emit_proj(j, sb, f):
            ss = slice(sb * 128, (sb + 1) * 128)
            sf = slice(f * 512, (f + 1) * 512)
            yp = psp.tile([128, 512], f32, tag="mm512", bufs=2, name="yproj")
            for r in range(PAIRS):
                nc.tensor.matmul(
                    yp,
                    lhsT=cat[:, r, ss],
                    rhs=wp_sb[:, r, sf],
                    start=(r == 0),
                    stop=(r == PAIRS - 1),
                )
            ys = work.tile([128, 512], f32, tag="ys", bufs=3)
            nc.vector.tensor_copy(ys, yp)
            nc.sync.dma_start(out=y_d[ss, sf], in_=ys)

        # ---------------- interleaved emission ----------------
        def interleave(main, filler):
            """Emit main items with filler spread evenly between them."""
            if not main:
                for g in filler:
                    g()
                return
            k = len(filler) / (len(main) + 1)
            fi = 0
            for n, g in enumerate(main):
                g()
                want = int((n + 1) * k)
                while fi < min(want, len(filler)):
                    filler[fi]()
                    fi += 1
            while fi < len(filler):
                filler[fi]()
                fi += 1

        for r in range(PAIRS):
            emit_qk(0, r, "q")

        for j in range(SCH):
            # K(j)/V(j) must precede chunk-j diagonal blocks in program
            # order (Tile deps follow emission order).  Zip them into r0's
            # non-diagonal items; at j=0 they simply come first.
            nd0 = [
                lambda j=j, ti=ti: emit_attn_ti(j, 0, ti)
                for ti in range(4 * j)
            ]
            kv = [lambda j=j, r=r: emit_qk(j, r, "k") for r in range(PAIRS)]
            kv += [lambda j=j, ii=ii: emit_v(j, ii) for ii in range(4)]
            interleave(nd0, kv)
            # rest: r0 diagonal+norm, then r1..r3 complete
            main = [
                lambda j=j, ti=ti: emit_attn_ti(j, 0, ti)
                for ti in range(4 * j, 4 * j + 4)
            ]
            main.append(lambda j=j: emit_norm(j, 0))
            for r in range(1, PAIRS):
                for ti in range(4 * j + 4):
                    main.append(lambda j=j, r=r, ti=ti: emit_attn_ti(j, r, ti))
                main.append(lambda j=j, r=r: emit_norm(j, r))
            filler = []
            if j > 0:
                for sb in range(4 * (j - 1), 4 * j):
                    for f in range(2):
                        filler.append(
                            lambda j=j, sb=sb, f=f: emit_proj(j - 1, sb, f)
                        )
            if j + 1 < SCH:
                for r in range(PAIRS):
                    filler.append(lambda j=j, r=r: emit_qk(j + 1, r, "q"))
            interleave(main, filler)

        for sb in range(12, 16):
            for f in range(2):
                emit_proj(3, sb, f)

    nc.compile()
    return nc


def get_nc():
    if "nc" not in _CACHE:
        _CACHE["nc"] = _build()
    return _CACHE["nc"]


def prep_core_inputs(x, Wq, bq, Wk, bk, Wv, bv, Wp, core):
    """Pack the full-model inputs into one core's input map."""
    b, g = core // 2, core % 2
    heads = list(range(g * 8, g * 8 + 8))

    xa = np.zeros((9 * 128, S), np.float32)
    xa[:E] = x[b].T
    xa[E] = 1.0

    def pack_w(W):  # [H,E,D] -> local [E, 512] -> [128, 8, 512]
        Wl = np.concatenate([W[h] for h in heads], axis=1)
        return host_round(Wl.reshape(ET, 128, 512).transpose(1, 0, 2))

    wv_aug = np.zeros((9 * 128, 512), np.float32)
    wv_aug[:E] = np.concatenate([Wv[h] for h in heads], axis=1)
    wv_aug[E] = np.concatenate([bv[h] for h in heads])

    wp_l = host_round(
        Wp[g * 512 : (g + 1) * 512].reshape(PAIRS, 128, E).transpose(1, 0, 2)
    )

    def pack_b(bias):
        return np.stack(
            [
                np.concatenate([bias[heads[2 * r]], bias[heads[2 * r + 1]]])
                for r in range(PAIRS)
            ],
            axis=1,
        ).astype(np.float32)

    p = np.arange(128)[:, None, None]
    v = np.arange(4)[None, :, None]
    c = np.arange(512)[None, None, :]
    mask = host_round((c >= p + 128 * v).astype(np.float32))

    return {
        "xt": host_round(xa.reshape(9, 128, S)),
        "wq": pack_w(Wq),
        "wk": pack_w(Wk),
        "wv": host_round(wv_aug.reshape(9, 128, 512).transpose(1, 0, 2)),
        "wp": wp_l,
        "bq": pack_b(bq),
        "bk": pack_b(bk),
        "mask": mask,
    }


def kernel(**inputs):
    from concourse.bass_utils import run_bass_kernel_spmd

    args = {k: np.asarray(v, np.float32) for k, v in inputs.items()}
    nc = get_nc()
    in_maps = [
        prep_core_inputs(
            args["x"], args["Wq"], args["bq"], args["Wk"], args["bk"],
            args["Wv"], args["bv"], args["Wp"], c,
        )
        for c in range(NCORES)
    ]
    res = run_bass_kernel_spmd(nc, in_maps, core_ids=list(range(NCORES)))
    parts = [r["y"] for r in res.results]
    out = np.stack([parts[2 * b] + parts[2 * b + 1] for b in range(B)])
    return (out + args["bp"][None, None, :]).astype(np.float32)
